# revision 13
# baseline (speedup 1.0000x reference)
"""KGramEmbeddingMLP on 8 TRN2 NeuronCores.

Model: one-hot context [256, 8*50257] -> embedding lookup -> MLP
512->1024->1024 (silu) -> vocab head 1024->50257.

The context is exactly one-hot per (batch, k) slot, so the embedding
"matmul" is a gather: indices are extracted host-side (argmax) and the
embedded activations x^T [512, 256] (bf16, 256KB) are fed directly to
every core.  That removes the 26MB/core one-hot streaming and the
entire phase-1 TensorE work of the dense formulation.

Device program (identical on all 8 cores, no collectives):
  - PE warmup: dummy matmuls burn the cold-clock (1.2GHz) activity
    window while the gating input DMAs are in flight.
  - MLP replicated over the full batch, computed transposed
    ([hidden-tile, batch]); W1/W2 packed m-major and loaded in slices
    so h1 starts as soon as x^T + the first W1 slice land.
  - Head tensor-parallel over vocab, TRANSPOSED: core c computes
    logits^T[shard, 256] = W3s^T @ h2 + b3s, streaming the 13.1MB W3
    shard through SBUF in column chunks that overlap the TensorE chunk
    loop.  The transposed orientation makes b3 a per-partition scalar
    (no [128 x VS] bias broadcast needed) and W3 the stationary
    operand.  The host transposes the [VS, 256] shard outputs back.

dtypes: all matmul operands bf16 (embedding values exact in bf16 cast),
PSUM f32, logits stored bf16 and upcast host-side (output rel-err
budget 2e-2; bf16 store adds ~1e-3).

Roofline per core: head PE 50*8*256 cycles ~ 42.7us @ 2.4GHz; MLP PE
~10.2us; DMA ~19.4MB @ 358GB/s ~ 54us -- both paths ~60-66us plus
~10us fixed framework pre/postamble.
"""

import numpy as np
import ml_dtypes

VOCAB = 50257
K = 8
EMBED = 64
HIDDEN = 1024
BATCH = 256
NCORES = 8

VP = 51200              # vocab padded to 8*6400
VS = VP // NCORES       # 6400 head rows per core (50 vocab tiles)
KT1 = (K * EMBED) // 128   # 4 contraction tiles for W1
KT2 = HIDDEN // 128        # 8 contraction tiles for W2 / W3
MT = HIDDEN // 128         # 8 hidden tiles
NVT = VS // 128            # 50 vocab tiles per core

# head vocab chunks: 12 x 512 + 1 x 256 (4 resp. 2 vocab tiles each)
CHUNKS = [(q * 512, 512) for q in range(12)] + [(6144, 256)]
NCH = len(CHUNKS)

BF16 = ml_dtypes.bfloat16

TRACE = False           # test.py sets this to capture a neuron profile
LAST_RESULT = None      # BassKernelResults from the most recent run

_compiled = {}


def _build():
    import concourse.mybir as mybir
    import concourse.tile as tile
    from concourse import bacc

    f32 = mybir.dt.float32
    bf16 = mybir.dt.bfloat16

    nc = bacc.Bacc(
        "TRN2", target_bir_lowering=False, debug=False, num_devices=NCORES
    )

    # host-packed inputs (partition dim first, fully contiguous free dim)
    # xw1a = x^T | first half of m-major W1; xw1b = second half of W1
    xw1a_d = nc.dram_tensor("xw1a", [128, KT1 * BATCH + 2048], bf16, kind="ExternalInput")
    xw1b_d = nc.dram_tensor("xw1b", [128, 2048], bf16, kind="ExternalInput")
    b1_d = nc.dram_tensor("b1t", [128, MT], f32, kind="ExternalInput")
    w2_d = nc.dram_tensor("w2", [128, KT2 * HIDDEN], bf16, kind="ExternalInput")
    b2_d = nc.dram_tensor("b2t", [128, MT], f32, kind="ExternalInput")
    # w3 packed chunk-major: [p, off3(q) + kk*w_q + j] = W3s[kk*128+p, off_q+j]
    w3_d = nc.dram_tensor("w3", [128, KT2 * VS], bf16, kind="ExternalInput")
    # b3 per vocab tile: [p, vt] = b3s[vt*128 + p]
    b3_d = nc.dram_tensor("b3", [128, NVT], f32, kind="ExternalInput")
    # out^T blocked per chunk (vocab-tile-major cols): host reassembles
    out_d = nc.dram_tensor("out", [NCH, 128, 4 * BATCH], bf16, kind="ExternalOutput")

    with tile.TileContext(nc) as tc:
        with (
            tc.tile_pool(name="const", bufs=1) as const,
            tc.tile_pool(name="w3s", bufs=4) as w3s,
            tc.tile_pool(name="mlp", bufs=1) as mlp,
            tc.tile_pool(name="head", bufs=4) as head,
            tc.tile_pool(name="psum_m", bufs=3, space="PSUM") as psum_m,
            tc.tile_pool(name="psum_w", bufs=1, space="PSUM") as psum_w,
            tc.tile_pool(name="psum_o", bufs=4, space="PSUM") as psum_o,
        ):
            # ---- PE clock warmup: burn the 1.2GHz activity window on
            # dummy matmuls while the gating DMAs are in flight --------
            wu_sb = mlp.tile([128, 128], bf16, tag="warm")
            nc.vector.memset(wu_sb[:], 0)
            wu_ps = psum_w.tile([128, 128], f32, tag="warm_ps")
            for _ in range(48):
                nc.tensor.matmul(
                    wu_ps[:], wu_sb[:], wu_sb[:], start=True, stop=True
                )

            # ---- loads, strict priority order on the sync ring ---------
            xw1a_sb = const.tile([128, KT1 * BATCH + 2048], bf16, tag="xw1a")
            nc.sync.dma_start(xw1a_sb[:], xw1a_d[:])
            xw1b_sb = const.tile([128, 2048], bf16, tag="xw1b")
            nc.sync.dma_start(xw1b_sb[:], xw1b_d[:])
            w2_sb = const.tile([128, KT2 * HIDDEN], bf16, tag="w2")
            W2Q = KT2 * HIDDEN // 4
            for s in range(4):
                nc.sync.dma_start(
                    w2_sb[:, s * W2Q:(s + 1) * W2Q], w2_d[:, s * W2Q:(s + 1) * W2Q]
                )
            b1_sb = const.tile([128, MT], f32, tag="b1")
            nc.scalar.dma_start(b1_sb[:], b1_d[:])
            b2_sb = const.tile([128, MT], f32, tag="b2")
            nc.scalar.dma_start(b2_sb[:], b2_d[:])
            b3_sb = const.tile([128, NVT], f32, tag="b3")
            nc.scalar.dma_start(b3_sb[:], b3_d[:])

            # ---- W3 chunk stream on the sync ring ----------------------
            w3_tiles = []
            off3 = 0
            for q, (off, w) in enumerate(CHUNKS):
                t = w3s.tile([128, KT2 * 512], bf16, tag="w3c")
                nc.sync.dma_start(t[:, :KT2 * w], w3_d[:, off3:off3 + KT2 * w])
                w3_tiles.append(t)
                off3 += KT2 * w

            def w1_slice(m, kk):
                if m < 4:
                    base = KT1 * BATCH + m * 512 + kk * 128
                    return xw1a_sb[:, base:base + 128]
                base = (m - 4) * 512 + kk * 128
                return xw1b_sb[:, base:base + 128]

            # ---- MLP (full batch, transposed activations) --------------
            h1t = []
            for m in range(MT):
                ps = psum_m.tile([128, BATCH], f32, tag="ps_mlp")
                for kk in range(KT1):
                    nc.tensor.matmul(
                        ps[:],
                        w1_slice(m, kk),
                        xw1a_sb[:, kk * BATCH:(kk + 1) * BATCH],
                        start=(kk == 0),
                        stop=(kk == KT1 - 1),
                    )
                t = mlp.tile([128, BATCH], bf16, tag=f"h1_{m}")
                nc.scalar.activation(
                    t[:], ps[:],
                    mybir.ActivationFunctionType.Silu,
                    bias=b1_sb[:, m:m + 1],
                )
                h1t.append(t)

            h2t = []
            for m in range(MT):
                ps = psum_m.tile([128, BATCH], f32, tag="ps_mlp")
                for kk in range(KT2):
                    nc.tensor.matmul(
                        ps[:],
                        w2_sb[:, m * HIDDEN + kk * 128:m * HIDDEN + (kk + 1) * 128],
                        h1t[kk][:],
                        start=(kk == 0),
                        stop=(kk == KT2 - 1),
                    )
                t = mlp.tile([128, BATCH], bf16, tag=f"h2_{m}")
                nc.scalar.activation(
                    t[:], ps[:],
                    mybir.ActivationFunctionType.Silu,
                    bias=b2_sb[:, m:m + 1],
                )
                h2t.append(t)

            # ---- head: logits^T[shard] = W3s^T @ h2 + b3s --------------
            # W3 tiles are the stationary operand; bias is a per-partition
            # scalar in the transposed orientation
            for q, (off, w) in enumerate(CHUNKS):
                wt = w3_tiles[q]
                nvt = w // 128
                osb = head.tile([128, 4 * BATCH], bf16, tag="osb")
                for vt in range(nvt):
                    ps = psum_o.tile([128, BATCH], f32, tag="ps_out")
                    for kk in range(KT2):
                        nc.tensor.matmul(
                            ps[:],
                            wt[:, kk * w + vt * 128:kk * w + (vt + 1) * 128],
                            h2t[kk][:],
                            start=(kk == 0),
                            stop=(kk == KT2 - 1),
                        )
                    nc.vector.tensor_scalar_add(
                        osb[:, vt * BATCH:(vt + 1) * BATCH], ps[:],
                        b3_sb[:, off // 128 + vt:off // 128 + vt + 1],
                    )
                nc.scalar.dma_start(out_d[q][:, :nvt * BATCH], osb[:, :nvt * BATCH])

    nc.compile()
    return nc


def _get_nc():
    if "nc" not in _compiled:
        _compiled["nc"] = _build()
    return _compiled["nc"]


def _prep_inputs(context_flat, embed_w, W1, b1, W2, b2, W3, b3):
    # one-hot -> indices -> gather (exact: context is one-hot per slot)
    ctx3 = np.asarray(context_flat).reshape(BATCH, K, VOCAB)
    idx = np.argmax(ctx3, axis=-1)                       # [B, K]
    emb = np.asarray(embed_w, np.float32)[idx]           # [B, K, EMBED] f32
    # x^T [K*EMBED, BATCH], packed as [128, KT1*BATCH]
    xT = np.ascontiguousarray(
        emb.reshape(BATCH, K * EMBED).T.astype(BF16)
    )
    xt_p = np.ascontiguousarray(
        xT.reshape(KT1, 128, BATCH).transpose(1, 0, 2)
    ).reshape(128, KT1 * BATCH)

    def pack_w(wm, kt):
        # m-major: [p, m*(kt*128) + kk*128 + j] = W[kk*128+p, m*128+j]
        w = np.asarray(wm, np.float32).astype(BF16)      # [kt*128, MT*128]
        return np.ascontiguousarray(
            w.reshape(kt, 128, MT, 128).transpose(1, 2, 0, 3)
        ).reshape(128, kt * MT * 128)

    w1_p = pack_w(W1, KT1)
    w2_p = pack_w(W2, KT2)
    xw1a = np.ascontiguousarray(np.concatenate([xt_p, w1_p[:, :2048]], axis=1))
    xw1b = np.ascontiguousarray(w1_p[:, 2048:])
    b1t = np.ascontiguousarray(np.asarray(b1, np.float32).reshape(MT, 128).T)
    b2t = np.ascontiguousarray(np.asarray(b2, np.float32).reshape(MT, 128).T)

    w3_p = np.zeros((HIDDEN, VP), BF16)
    w3_p[:, :VOCAB] = np.asarray(W3, np.float32).astype(BF16)
    b3_p = np.zeros(VP, np.float32)
    b3_p[:VOCAB] = np.asarray(b3, np.float32)

    in_maps = []
    for c in range(NCORES):
        w3s = w3_p[:, c * VS:(c + 1) * VS]               # [1024, 6400]
        # chunk-major pack: [128, sum_q 8*w_q]
        w3_pk = np.empty((128, KT2 * VS), BF16)
        off3 = 0
        for off, w in CHUNKS:
            blk = w3s[:, off:off + w].reshape(KT2, 128, w).transpose(1, 0, 2)
            w3_pk[:, off3:off3 + KT2 * w] = blk.reshape(128, KT2 * w)
            off3 += KT2 * w
        b3_pk = np.ascontiguousarray(
            b3_p[c * VS:(c + 1) * VS].reshape(NVT, 128).T
        )
        in_maps.append({
            "xw1a": xw1a,
            "xw1b": xw1b,
            "b1t": b1t,
            "w2": w2_p,
            "b2t": b2t,
            "w3": np.ascontiguousarray(w3_pk),
            "b3": b3_pk,
        })
    return in_maps


def kernel(**inputs):
    global LAST_RESULT
    from concourse import bass_utils

    nc = _get_nc()
    in_maps = _prep_inputs(**inputs)
    res = bass_utils.run_bass_kernel_spmd(
        nc, in_maps, core_ids=list(range(NCORES)), trace=TRACE
    )
    LAST_RESULT = res
    full = np.empty((BATCH, VP), np.float32)
    for c in range(NCORES):
        o = res.results[c]["out"].astype(np.float32)     # [NCH, 128, 4*BATCH]
        for q, (off, w) in enumerate(CHUNKS):
            nvt = w // 128
            blk = o[q][:, :nvt * BATCH].reshape(128, nvt, BATCH)
            arr = blk.transpose(1, 0, 2).reshape(nvt * 128, BATCH)
            full[:, c * VS + off:c * VS + off + w] = arr.T
    return np.ascontiguousarray(full[:, :VOCAB])


# revision 16
# speedup vs baseline: 1.0079x; 1.0079x over previous
"""KGramEmbeddingMLP on 8 TRN2 NeuronCores.

Model: one-hot context [256, 8*50257] -> embedding lookup -> MLP
512->1024->1024 (silu) -> vocab head 1024->50257.

The context is exactly one-hot per (batch, k) slot, so the embedding
"matmul" is a gather: indices are extracted host-side (argmax) and the
embedded activations x^T [512, 256] (bf16, 256KB) are fed directly to
every core.  That removes the 26MB/core one-hot streaming and the
entire phase-1 TensorE work of the dense formulation.

Device program (identical on all 8 cores, no collectives):
  - PE warmup: dummy matmuls burn the cold-clock (1.2GHz) activity
    window while the gating input DMAs are in flight.
  - MLP replicated over the full batch, computed transposed
    ([hidden-tile, batch]); W1/W2 packed m-major and loaded in slices
    so h1 starts as soon as x^T + the first W1 slice land.
  - Head tensor-parallel over vocab, TRANSPOSED: core c computes
    logits^T[shard, 256] = W3s^T @ h2 + b3s, streaming the 13.1MB W3
    shard through SBUF in column chunks that overlap the TensorE chunk
    loop.  The transposed orientation makes b3 a per-partition scalar
    (no [128 x VS] bias broadcast needed) and W3 the stationary
    operand.  The host transposes the [VS, 256] shard outputs back.

dtypes: all matmul operands bf16 (embedding values exact in bf16 cast),
PSUM f32, logits stored bf16 and upcast host-side (output rel-err
budget 2e-2; bf16 store adds ~1e-3).

Roofline per core: head PE 50*8*256 cycles ~ 42.7us @ 2.4GHz; MLP PE
~10.2us; DMA ~19.4MB @ 358GB/s ~ 54us -- both paths ~60-66us plus
~10us fixed framework pre/postamble.
"""

import numpy as np
import ml_dtypes

VOCAB = 50257
K = 8
EMBED = 64
HIDDEN = 1024
BATCH = 256
NCORES = 8

VP = 51200              # vocab padded to 8*6400
VS = VP // NCORES       # 6400 head rows per core (50 vocab tiles)
KT1 = (K * EMBED) // 128   # 4 contraction tiles for W1
KT2 = HIDDEN // 128        # 8 contraction tiles for W2 / W3
MT = HIDDEN // 128         # 8 hidden tiles
NVT = VS // 128            # 50 vocab tiles per core

# head vocab chunks: 12 x 512 + 1 x 256 (4 resp. 2 vocab tiles each)
CHUNKS = [(q * 512, 512) for q in range(12)] + [(6144, 256)]
NCH = len(CHUNKS)

BF16 = ml_dtypes.bfloat16

TRACE = False           # test.py sets this to capture a neuron profile
LAST_RESULT = None      # BassKernelResults from the most recent run

_compiled = {}


def _build():
    import concourse.mybir as mybir
    import concourse.tile as tile
    from concourse import bacc

    f32 = mybir.dt.float32
    bf16 = mybir.dt.bfloat16

    nc = bacc.Bacc(
        "TRN2", target_bir_lowering=False, debug=False, num_devices=NCORES
    )

    # host-packed inputs (partition dim first, fully contiguous free dim)
    # xw1a = x^T | m-major W1 (loaded in slices: the first h1 tile only
    # needs x^T + W1's m0 slice)
    xw1a_d = nc.dram_tensor("xw1a", [128, KT1 * BATCH + 2048], bf16, kind="ExternalInput")
    xw1b_d = nc.dram_tensor("xw1b", [128, 2048], bf16, kind="ExternalInput")
    b1_d = nc.dram_tensor("b1t", [128, MT], f32, kind="ExternalInput")
    w2_d = nc.dram_tensor("w2", [128, KT2 * HIDDEN], bf16, kind="ExternalInput")
    b2_d = nc.dram_tensor("b2t", [128, MT], f32, kind="ExternalInput")
    # w3 packed chunk-major: [p, off3(q) + kk*w_q + j] = W3s[kk*128+p, off_q+j]
    w3_d = nc.dram_tensor("w3", [128, KT2 * VS], bf16, kind="ExternalInput")
    # b3 per vocab tile: [p, vt] = b3s[vt*128 + p]
    b3_d = nc.dram_tensor("b3", [128, NVT], f32, kind="ExternalInput")
    # out^T blocked per chunk (vocab-tile-major cols): host reassembles
    out_d = nc.dram_tensor("out", [NCH, 128, 4 * BATCH], bf16, kind="ExternalOutput")

    with tile.TileContext(nc) as tc:
        with (
            tc.tile_pool(name="const", bufs=1) as const,
            tc.tile_pool(name="w3s", bufs=6) as w3s,
            tc.tile_pool(name="mlp", bufs=1) as mlp,
            tc.tile_pool(name="head", bufs=6) as head,
            tc.tile_pool(name="psum_m", bufs=3, space="PSUM") as psum_m,
            tc.tile_pool(name="psum_w", bufs=1, space="PSUM") as psum_w,
            tc.tile_pool(name="psum_o", bufs=4, space="PSUM") as psum_o,
        ):
            # ---- PE clock warmup: burn the 1.2GHz activity window on
            # dummy matmuls while the gating DMAs are in flight --------
            wu_sb = mlp.tile([128, 128], bf16, tag="warm")
            nc.vector.memset(wu_sb[:], 0)
            wu_ps = psum_w.tile([128, 128], f32, tag="warm_ps")
            for _ in range(48):
                nc.tensor.matmul(
                    wu_ps[:], wu_sb[:], wu_sb[:], start=True, stop=True
                )

            # ---- loads, strict priority order on the sync ring ---------
            # first DMA is the minimal h1-m0 gate: x^T + W1's m0 slice
            XA = KT1 * BATCH + 512
            xw1a_sb = const.tile([128, KT1 * BATCH + 2048], bf16, tag="xw1a")
            nc.sync.dma_start(xw1a_sb[:, :XA], xw1a_d[:, :XA])
            nc.sync.dma_start(xw1a_sb[:, XA:], xw1a_d[:, XA:])
            xw1b_sb = const.tile([128, 2048], bf16, tag="xw1b")
            nc.sync.dma_start(xw1b_sb[:], xw1b_d[:])
            w2_sb = const.tile([128, KT2 * HIDDEN], bf16, tag="w2")
            W2Q = KT2 * HIDDEN // 4
            for s in range(4):
                nc.sync.dma_start(
                    w2_sb[:, s * W2Q:(s + 1) * W2Q], w2_d[:, s * W2Q:(s + 1) * W2Q]
                )
            b1_sb = const.tile([128, MT], f32, tag="b1")
            nc.scalar.dma_start(b1_sb[:], b1_d[:])
            b2_sb = const.tile([128, MT], f32, tag="b2")
            nc.scalar.dma_start(b2_sb[:], b2_d[:])
            b3_sb = const.tile([128, NVT], f32, tag="b3")
            nc.scalar.dma_start(b3_sb[:], b3_d[:])

            # ---- W3 chunk stream on the sync ring ----------------------
            w3_tiles = []
            off3 = 0
            for q, (off, w) in enumerate(CHUNKS):
                t = w3s.tile([128, KT2 * 512], bf16, tag="w3c")
                nc.sync.dma_start(t[:, :KT2 * w], w3_d[:, off3:off3 + KT2 * w])
                w3_tiles.append(t)
                off3 += KT2 * w

            def w1_slice(m, kk):
                if m < 4:
                    base = KT1 * BATCH + m * 512 + kk * 128
                    return xw1a_sb[:, base:base + 128]
                base = (m - 4) * 512 + kk * 128
                return xw1b_sb[:, base:base + 128]

            # ---- MLP (full batch, transposed activations) --------------
            h1t = []
            for m in range(MT):
                ps = psum_m.tile([128, BATCH], f32, tag="ps_mlp")
                for kk in range(KT1):
                    nc.tensor.matmul(
                        ps[:],
                        w1_slice(m, kk),
                        xw1a_sb[:, kk * BATCH:(kk + 1) * BATCH],
                        start=(kk == 0),
                        stop=(kk == KT1 - 1),
                    )
                t = mlp.tile([128, BATCH], bf16, tag=f"h1_{m}")
                nc.scalar.activation(
                    t[:], ps[:],
                    mybir.ActivationFunctionType.Silu,
                    bias=b1_sb[:, m:m + 1],
                )
                h1t.append(t)

            h2t = []
            for m in range(MT):
                ps = psum_m.tile([128, BATCH], f32, tag="ps_mlp")
                for kk in range(KT2):
                    nc.tensor.matmul(
                        ps[:],
                        w2_sb[:, m * HIDDEN + kk * 128:m * HIDDEN + (kk + 1) * 128],
                        h1t[kk][:],
                        start=(kk == 0),
                        stop=(kk == KT2 - 1),
                    )
                t = mlp.tile([128, BATCH], bf16, tag=f"h2_{m}")
                nc.scalar.activation(
                    t[:], ps[:],
                    mybir.ActivationFunctionType.Silu,
                    bias=b2_sb[:, m:m + 1],
                )
                h2t.append(t)

            # ---- head: logits^T[shard] = W3s^T @ h2 + b3s --------------
            # W3 tiles are the stationary operand; bias is a per-partition
            # scalar in the transposed orientation
            for q, (off, w) in enumerate(CHUNKS):
                wt = w3_tiles[q]
                nvt = w // 128
                osb = head.tile([128, 4 * BATCH], bf16, tag="osb")
                for vt in range(nvt):
                    ps = psum_o.tile([128, BATCH], f32, tag="ps_out")
                    for kk in range(KT2):
                        nc.tensor.matmul(
                            ps[:],
                            wt[:, kk * w + vt * 128:kk * w + (vt + 1) * 128],
                            h2t[kk][:],
                            start=(kk == 0),
                            stop=(kk == KT2 - 1),
                        )
                    nc.vector.tensor_scalar_add(
                        osb[:, vt * BATCH:(vt + 1) * BATCH], ps[:],
                        b3_sb[:, off // 128 + vt:off // 128 + vt + 1],
                    )
                nc.scalar.dma_start(out_d[q][:, :nvt * BATCH], osb[:, :nvt * BATCH])

    nc.compile()
    return nc


def _get_nc():
    if "nc" not in _compiled:
        _compiled["nc"] = _build()
    return _compiled["nc"]


def _prep_inputs(context_flat, embed_w, W1, b1, W2, b2, W3, b3):
    # one-hot -> indices -> gather (exact: context is one-hot per slot)
    ctx3 = np.asarray(context_flat).reshape(BATCH, K, VOCAB)
    idx = np.argmax(ctx3, axis=-1)                       # [B, K]
    emb = np.asarray(embed_w, np.float32)[idx]           # [B, K, EMBED] f32
    # x^T [K*EMBED, BATCH], packed as [128, KT1*BATCH]
    xT = np.ascontiguousarray(
        emb.reshape(BATCH, K * EMBED).T.astype(BF16)
    )
    xt_p = np.ascontiguousarray(
        xT.reshape(KT1, 128, BATCH).transpose(1, 0, 2)
    ).reshape(128, KT1 * BATCH)

    def pack_w(wm, kt):
        # m-major: [p, m*(kt*128) + kk*128 + j] = W[kk*128+p, m*128+j]
        w = np.asarray(wm, np.float32).astype(BF16)      # [kt*128, MT*128]
        return np.ascontiguousarray(
            w.reshape(kt, 128, MT, 128).transpose(1, 2, 0, 3)
        ).reshape(128, kt * MT * 128)

    w1_p = pack_w(W1, KT1)
    w2_p = pack_w(W2, KT2)
    xw1a = np.ascontiguousarray(np.concatenate([xt_p, w1_p[:, :2048]], axis=1))
    xw1b = np.ascontiguousarray(w1_p[:, 2048:])
    b1t = np.ascontiguousarray(np.asarray(b1, np.float32).reshape(MT, 128).T)
    b2t = np.ascontiguousarray(np.asarray(b2, np.float32).reshape(MT, 128).T)

    w3_p = np.zeros((HIDDEN, VP), BF16)
    w3_p[:, :VOCAB] = np.asarray(W3, np.float32).astype(BF16)
    b3_p = np.zeros(VP, np.float32)
    b3_p[:VOCAB] = np.asarray(b3, np.float32)

    in_maps = []
    for c in range(NCORES):
        w3s = w3_p[:, c * VS:(c + 1) * VS]               # [1024, 6400]
        # chunk-major pack: [128, sum_q 8*w_q]
        w3_pk = np.empty((128, KT2 * VS), BF16)
        off3 = 0
        for off, w in CHUNKS:
            blk = w3s[:, off:off + w].reshape(KT2, 128, w).transpose(1, 0, 2)
            w3_pk[:, off3:off3 + KT2 * w] = blk.reshape(128, KT2 * w)
            off3 += KT2 * w
        b3_pk = np.ascontiguousarray(
            b3_p[c * VS:(c + 1) * VS].reshape(NVT, 128).T
        )
        in_maps.append({
            "xw1a": xw1a,
            "xw1b": xw1b,
            "b1t": b1t,
            "w2": w2_p,
            "b2t": b2t,
            "w3": np.ascontiguousarray(w3_pk),
            "b3": b3_pk,
        })
    return in_maps


def kernel(**inputs):
    global LAST_RESULT
    from concourse import bass_utils

    nc = _get_nc()
    in_maps = _prep_inputs(**inputs)
    res = bass_utils.run_bass_kernel_spmd(
        nc, in_maps, core_ids=list(range(NCORES)), trace=TRACE
    )
    LAST_RESULT = res
    full = np.empty((BATCH, VP), np.float32)
    for c in range(NCORES):
        o = res.results[c]["out"].astype(np.float32)     # [NCH, 128, 4*BATCH]
        for q, (off, w) in enumerate(CHUNKS):
            nvt = w // 128
            blk = o[q][:, :nvt * BATCH].reshape(128, nvt, BATCH)
            arr = blk.transpose(1, 0, 2).reshape(nvt * 128, BATCH)
            full[:, c * VS + off:c * VS + off + w] = arr.T
    return np.ascontiguousarray(full[:, :VOCAB])
